# revision 36
# baseline (speedup 1.0000x reference)
"""DGCNN (nn_DGCNN_type1) Trainium2 Bass kernel — self-contained.

Strategy: data-parallel over the 128 graphs, 16 per NeuronCore across 8 cores.
Per graph: f16 kNN-score matmul (s = 2*f@f.T - |f_j|^2, row-constant dropped)
+ DVE max8/max_index/match_replace top-16 on f16 scores; edge MLP via the
q-trick: q = W_bot^T @ feat computed once per conv, transposed on-chip to a
node-major f16 table, and gathered per edge with SBUF-source dma_gather
(SBUF->SBUF, no HBM traffic); an identity matmul folds the gathered columns
into PSUM on top of the broadcast W_mod matmul. k-max folded through the
monotone LeakyReLU as a PSUM-side reduce; lin1 (hidden shared across both
output halves) + global-max-pool + MLP head on-chip.
"""

import numpy as np
import concourse.bacc as bacc
import concourse.mybir as mybir
from concourse.tile import TileContext
from concourse.masks import make_identity

F32, F16, I16, U16 = (mybir.dt.float32, mybir.dt.float16, mybir.dt.int16,
                      mybir.dt.uint16)
F32R = mybir.dt.float32r
AF = mybir.ActivationFunctionType
ALU = mybir.AluOpType
AX = mybir.AxisListType

N = 512
K = 16
# timing-bisect knobs (default = full kernel)
SKIP_TOPK = False
SKIP_GATHER = False
NUM_QUEUES = 1     # SWDGE queues; gathers round-robin across them
NCHUNK = N // 128  # 4 row-chunks for the NxN score matrix


def make_nc():
    return bacc.Bacc(num_swdge_queues=NUM_QUEUES)


def host_prep(inputs, G, core):
    """Build the per-core in_map (numpy only: layout/dtype prep, no model math)."""
    f16 = np.float16
    x, pos, tq = inputs["x"], inputs["pos"], inputs["tq"]
    B_all = x.shape[0] // N
    xx = np.concatenate([tq, x, pos], axis=1).reshape(B_all, N, 5).astype(np.float32)
    sl = slice(core * G, (core + 1) * G)
    feat5h = np.ascontiguousarray(xx[sl].transpose(0, 2, 1)).astype(f16)

    w1a, w1b = inputs["w1a"], inputs["w1b"]
    w2a, w2b = inputs["w2a"], inputs["w2b"]
    wl1 = inputs["wl1"]

    return {
        "feat5h": feat5h,
        "w1mod": np.ascontiguousarray((w1a[0:5] - w1a[5:10]).astype(f16)),
        "w1bot": np.ascontiguousarray(w1a[5:10].astype(f16)),
        "w1b": np.ascontiguousarray(w1b.astype(f16)),
        "b1a": inputs["b1a"].reshape(64, 1).astype(np.float32),
        "b1b2": np.tile(inputs["b1b"], 2).reshape(128, 1).astype(np.float32),
        "w2mod": np.ascontiguousarray((w2a[0:64] - w2a[64:128]).astype(f16)),
        "w2bot": np.ascontiguousarray(w2a[64:128].astype(f16)),
        "w2b": np.ascontiguousarray(w2b.astype(f16)),
        "b2a": inputs["b2a"].reshape(128, 1).astype(np.float32),
        "b2b2": np.tile(inputs["b2b"], 2).reshape(128, 1).astype(np.float32),
        "wl1xx": np.ascontiguousarray(wl1[0:5].astype(f16)),
        "wl1x1": np.ascontiguousarray(wl1[5:69].astype(f16)),
        "wl1x2": np.ascontiguousarray(wl1[69:133].astype(f16)),
        "bl1c": np.ascontiguousarray(inputs["bl1"].reshape(4, 128).T.astype(np.float32)),
        "wl2": np.ascontiguousarray(inputs["wl2"].astype(f16).reshape(4, 128, 256).transpose(1, 0, 2)),
        "bl2c": np.ascontiguousarray(inputs["bl2"].reshape(2, 128).T.astype(np.float32)),
        "wm1": np.ascontiguousarray(inputs["wm1"].astype(f16).reshape(2, 128, 128).transpose(1, 0, 2)),
        "bm1": inputs["bm1"].reshape(128, 1).astype(np.float32),
        "wm2": np.ascontiguousarray(inputs["wm2"].astype(f16)),
        "bm2": inputs["bm2"].reshape(3, 1).astype(np.float32),
        "repl16": np.ascontiguousarray(np.tile(np.eye(16), 8).astype(f16)),
    }


def declare_io(nc, G):
    t = {}
    def inp(name, shape, dt):
        t[name] = nc.dram_tensor(name, shape, dt, kind="ExternalInput")
    inp("feat5h", [G, 5, N], F16)
    inp("w1mod", [5, 64], F16); inp("w1bot", [5, 64], F16)
    inp("w1b", [64, 64], F16); inp("b1a", [64, 1], F32); inp("b1b2", [128, 1], F32)
    inp("w2mod", [64, 128], F16); inp("w2bot", [64, 128], F16)
    inp("w2b", [128, 64], F16); inp("b2a", [128, 1], F32); inp("b2b2", [128, 1], F32)
    inp("wl1xx", [5, N], F16); inp("wl1x1", [64, N], F16); inp("wl1x2", [64, N], F16)
    inp("bl1c", [128, 4], F32); inp("wl2", [128, 4, 256], F16); inp("bl2c", [128, 2], F32)
    inp("wm1", [128, 2, 128], F16); inp("bm1", [128, 1], F32)
    inp("wm2", [128, 3], F16); inp("bm2", [3, 1], F32)
    inp("repl16", [16, 128], F16)
    t["o"] = nc.dram_tensor("o", [3, G], F32, kind="ExternalOutput")
    return t


def build(nc, G, reps=1):
    t = declare_io(nc, G)
    with TileContext(nc) as tc:
        _build_body(nc, tc, t, G, reps)
    nc.compile()
    return t


def _build_body(nc, tc, t, G, reps=1):
    sbw = tc.alloc_tile_pool(name="sbw", bufs=1)          # persistent
    sb = tc.alloc_tile_pool(name="sb", bufs=4)            # rotating tiles
    sb3 = tc.alloc_tile_pool(name="sb3", bufs=4)          # h1 rotation
    sbg = tc.alloc_tile_pool(name="sbg", bufs=8)          # in-flight gather outs
    sbl = tc.alloc_tile_pool(name="sbl", bufs=4)          # lin1 hidden cache
    ps_nd = tc.alloc_tile_pool(name="ps_nd", bufs=1, space="PSUM")   # 1 bank
    ps_a1 = tc.alloc_tile_pool(name="ps_a1", bufs=2, space="PSUM")   # 4 banks
    ps_a2 = tc.alloc_tile_pool(name="ps_a2", bufs=1, space="PSUM")   # 2 banks
    ps_sm = tc.alloc_tile_pool(name="ps_sm", bufs=1, space="PSUM")   # 1 bank

    # ---- persistent weight tiles ----
    w = {}
    for name in ["w1mod", "w1bot", "w1b", "w2mod", "w2bot", "w2b",
                 "wl1xx", "wl1x1", "wl1x2", "wl2", "wm1", "wm2", "repl16"]:
        w[name] = sbw.tile(list(t[name].shape), F16, tag=name, name='w_'+name)
        nc.sync.dma_start(out=w[name][:], in_=t[name][:])
    for name in ["b1a", "b1b2", "b2a", "b2b2", "bl1c", "bl2c", "bm1", "bm2"]:
        w[name] = sbw.tile(list(t[name].shape), F32, tag=name, name='b_'+name)
        nc.sync.dma_start(out=w[name][:], in_=t[name][:])
    ident = sbw.tile([128, 128], F16, tag="ident")
    make_identity(nc, ident[:])
    ones = sbw.tile([64, 1], F16, tag="ones")
    nc.gpsimd.memset(ones[:], 1.0)
    onesrow = sbw.tile([1, N], F16, tag="onesrow")
    nc.gpsimd.memset(onesrow[:], 1.0)

    # persistent gather-index tiles (all 128 rows hold the wrapped idxT,
    # replicated per 16-partition gpsimd core group)
    NIDX_SLOTS = 4
    idx_tiles = []
    for s in range(NIDX_SLOTS):
        it = sbw.tile([128, N], I16, tag=f"idxs{s}", name=f"idxs{s}")
        nc.gpsimd.memset(it[:], 0)
        idx_tiles.append(it)
    idx_slot = [0]
    gq = [0]  # round-robin SWDGE queue cursor for the gathers

    Gt_lo = sbw.tile([128, G], F32, tag="gtlo")
    Gt_hi = sbw.tile([128, G], F32, tag="gthi")

    rep_ctx = tc.For_i(0, reps, 1) if reps > 1 else None
    if rep_ctx is not None:
        rep_ctx.__enter__()

    def conv_A(d, Fmid, wmod, wbot, ba, fill_B, featTh):
        """DynamicEdgeConv phase A: scores, topk, index replication, q
        table, and the 4 SBUF-source gathers (launched, not consumed).
        Returns state for conv_B. Emitting other work between A and B
        hides the gather DMA latency."""
        dp = d + 1
        B = sb.tile([65, N], F16, tag="Btile")
        fill_B(B)                              # fills B[0:d, :] (f16)
        sc = sb.tile([65, N], F16, tag="sctile")
        nc.vector.tensor_scalar_mul(sc[0:d, :], B[0:d, :], 2.0)
        F2 = sb.tile([64, N], F16, tag="F2tile")
        nc.scalar.activation(F2[0:d, :], B[0:d, :], AF.Square)
        sqp = ps_sm.tile([1, N], F32, tag="sm")
        nc.tensor.matmul(out=sqp[:], lhsT=ones[0:d, :], rhs=F2[0:d, :],
                         start=True, stop=True)
        if d % 32 == 0:
            nc.gpsimd.memset(sc[d:d + 1, :], 1.0)
            nc.scalar.activation(B[d:d + 1, :], sqp[:], AF.Identity, scale=-1.0)
        else:
            nc.sync.dma_start(out=sc[d:d + 1, :], in_=onesrow[:])
            sqtmp = sb.tile([1, N], F16, tag="sqtmp")
            nc.scalar.activation(sqtmp[:], sqp[:], AF.Identity, scale=-1.0)
            nc.sync.dma_start(out=B[d:d + 1, :], in_=sqtmp[:])

        # ---- negd2 chunks + topk -> replicated idx tile ----
        idxs = idx_tiles[idx_slot[0] % NIDX_SLOTS]
        idx_slot[0] += 1
        idxTp = ps_sm.tile([16, N], F16, tag="sm")
        for c in range(NCHUNK):
            nd_p = ps_nd.tile([128, N], F32, tag="ndp")
            nc.tensor.matmul(out=nd_p[:], lhsT=sc[0:dp, 128 * c:128 * (c + 1)],
                             rhs=B[0:dp, :], start=True, stop=True)
            nd = sb.tile([128, N], F16, tag="ndsb")
            nc.scalar.activation(nd[:], nd_p[:], AF.Copy)
            maxv = sb.tile([128, 16], F16, tag="maxv")
            maxi = sb.tile([128, 16], U16, tag="maxi")
            if not SKIP_TOPK:
                nc.vector.max(out=maxv[:, 0:8], in_=nd[:])
                nc.vector.max_index(out=maxi[:, 0:8], in_max=maxv[:, 0:8], in_values=nd[:])
                nc.vector.match_replace(out=nd[:], in_to_replace=maxv[:, 0:8],
                                        in_values=nd[:], imm_value=-30000.0)
                nc.vector.max(out=maxv[:, 8:16], in_=nd[:])
                nc.vector.max_index(out=maxi[:, 8:16], in_max=maxv[:, 8:16], in_values=nd[:])
            else:
                nc.vector.memset(maxi[:], 0)
            mif = sb.tile([128, 16], F16, tag="mif")
            nc.vector.tensor_copy(mif[:], maxi[:])
            nc.tensor.transpose(out=idxTp[:, 128 * c:128 * (c + 1)], in_=mif[:],
                                identity=ident[:])
        # replicate wrapped idxT to all 8 gpsimd core groups via a
        # tiled-identity matmul: idxs[p, n] = idxT[p % 16, n]
        idxT16 = sb.tile([16, N], F16, tag="idxT16")
        nc.scalar.activation(idxT16[:], idxTp[:], AF.Copy)
        idxmm = ps_nd.tile([128, N], F32, tag="ndp")
        nc.tensor.matmul(out=idxmm[:], lhsT=w["repl16"][:], rhs=idxT16[:],
                         start=True, stop=True)
        nc.vector.tensor_copy(idxs[:], idxmm[:])

        # ---- q table: q = wbot^T @ feat, then node-major f16 for the
        # SBUF-source gather: T partition m%128, bytes [256*(m//128), +256)
        # hold node m's 128-row f16 q column ----
        qp = ps_nd.tile([128, N], F32, tag="ndp")
        nc.tensor.matmul(out=qp[0:Fmid, :], lhsT=wbot[:], rhs=featTh[:],
                         start=True, stop=True)
        qh = sb.tile([128, N], F16, tag="qtab")
        nc.scalar.activation(qh[0:Fmid, :], qp[0:Fmid, :], AF.Copy)
        if Fmid < 128:
            nc.gpsimd.memset(qh[Fmid:128, :], 0)
        Tp = ps_sm.tile([128, N], F16, tag="sm")
        for c in range(NCHUNK):
            nc.tensor.transpose(out=Tp[:, 128 * c:128 * (c + 1)],
                                in_=qh[:, 128 * c:128 * (c + 1)],
                                identity=ident[:])
        T = sb.tile([128, N], F16, tag="Ttab")
        nc.scalar.activation(T[:], Tp[:], AF.Copy)

        # ---- launch the 4 gathers (consumed later in conv_B) ----
        qgs = []
        for tt in range(4):
            qg = sbg.tile([128, 4 * N], F16, tag="qg")
            if SKIP_GATHER:
                for z in range(4):
                    nc.scalar.activation(qg[:, 512 * z:512 * (z + 1)], T[:],
                                         AF.Copy)
            else:
                nc.gpsimd.dma_gather(
                    out_ap=qg[:, None, :], in_ap=T[:],
                    idxs_ap=idxs[:, 128 * tt:128 * (tt + 1)],
                    num_idxs=4 * N, num_idxs_reg=4 * N, elem_size=128,
                    transpose=True, single_packet=False,
                    queue_num=gq[0] % NUM_QUEUES,
                    sbuf_tokens_per_rank=128, sbuf_free_dim_per_rank=256,
                    sbuf_free_dim_pad_per_rank=0, sbuf_byte_offset=0)
                gq[0] += 1
            qgs.append(qg)
        return dict(d=d, Fmid=Fmid, wmod=wmod, ba=ba, featTh=featTh, qgs=qgs)

    def conv_B(st, wsec, kact_sink):
        """DynamicEdgeConv phase B: fold gathered q into the broadcast
        W_mod matmul, Prelu, layer2, k-max. kact_sink(tt, kmx): kmx
        [128, 64] f32 pre-activation k-max, rows 64h+f = feature f of
        node group h; tile tt covers nodes [128*tt + 64*h, +64); the
        sink applies the output Prelu itself while scattering halves."""
        d, Fmid, wmod, ba, featTh, qgs = (st["d"], st["Fmid"], st["wmod"],
                                          st["ba"], st["featTh"], st["qgs"])
        for tt in range(4):
            qg = qgs[tt]
            a2 = ps_a2.tile([128, 1024], F32, tag="a2")
            for hh in range(2):
                a1 = ps_a1.tile([128, 1024], F32, tag="a1")
                for r in range(2):
                    qq = 2 * hh + r
                    c = 4 * tt + qq
                    nc.tensor.matmul(out=a1[0:Fmid, 512 * r:512 * (r + 1)],
                                     lhsT=wmod[:],
                                     rhs=featTh[:, 32 * c:32 * (c + 1), None]
                                     .to_broadcast([d, 32, K]),
                                     start=True, stop=False)
                    nc.tensor.matmul(out=a1[0:Fmid, 512 * r:512 * (r + 1)],
                                     lhsT=ident[:, 0:Fmid],
                                     rhs=qg[:, 512 * qq:512 * (qq + 1)],
                                     start=False, stop=True)
                h1 = sb3.tile([128, 1024], F16, tag="h1")
                nc.scalar.activation(h1[0:Fmid, :], a1[0:Fmid, :], AF.Prelu,
                                     bias=ba[:], alpha=0.01)
                for r in range(2):
                    nc.tensor.matmul(out=a2[64 * hh:64 * hh + 64, 512 * r:512 * (r + 1)],
                                     lhsT=wsec[:], rhs=h1[0:Fmid, 512 * r:512 * (r + 1)],
                                     start=True, stop=True)
            kmx = sb.tile([128, 64], F32, tag="kmx")
            nc.vector.tensor_reduce(out=kmx[:], in_=a2[:].rearrange(
                "p (n k) -> p n k", k=K), op=ALU.max, axis=AX.X)
            kact_sink(tt, kmx)

    def lin1_pool(g, f5h, x1f16, x2f16):
        hsbs = []
        for c in range(NCHUNK):
            hp = ps_a1.tile([128, 1024], F32, tag="a1")
            nc.tensor.matmul(out=hp[:, 0:N], lhsT=w["wl1xx"][:, 128 * c:128 * (c + 1)],
                             rhs=f5h[:], start=True, stop=False)
            nc.tensor.matmul(out=hp[:, 0:N], lhsT=w["wl1x1"][:, 128 * c:128 * (c + 1)],
                             rhs=x1f16[:], start=False, stop=False)
            nc.tensor.matmul(out=hp[:, 0:N], lhsT=w["wl1x2"][:, 128 * c:128 * (c + 1)],
                             rhs=x2f16[:], start=False, stop=True)
            hsb = sbl.tile([128, N], F16, tag="hsb")
            nc.scalar.activation(hsb[:], hp[:, 0:N], AF.Prelu,
                                 bias=w["bl1c"][:, c:c + 1], alpha=0.01)
            hsbs.append(hsb)
        for fo in range(2):
            h2p = ps_a2.tile([128, 1024], F32, tag="a2")
            for c in range(NCHUNK):
                nc.tensor.matmul(out=h2p[:, 0:N],
                                 lhsT=w["wl2"][:, c, 128 * fo:128 * (fo + 1)],
                                 rhs=hsbs[c][:], start=(c == 0), stop=(c == NCHUNK - 1))
            gt = Gt_lo if fo == 0 else Gt_hi
            nc.vector.tensor_reduce(out=gt[:, g:g + 1], in_=h2p[:, 0:N], op=ALU.max,
                                    axis=AX.X)

    # Software-pipelined graph loop: each conv's gathers (phase A) are
    # launched one work-unit ahead of their consumption (phase B), so the
    # SWDGE DMA latency hides under the neighbouring unit's compute.
    prev = None  # (stA2, sink2, lin1 args) for graph g-1
    for g in range(G):
        # ---- conv1 phase A ----
        f5h = sb.tile([5, N], F16, tag="f5h")
        nc.sync.dma_start(out=f5h[:], in_=t["feat5h"][g])

        def fill_B_conv1(B, g=g):
            nc.sync.dma_start(out=B[0:5, :], in_=t["feat5h"][g])
        x1f16 = sb.tile([64, N], F16, tag="x1f16")

        def sink1(tt, kmx, x1f16=x1f16):
            for h in range(2):
                cols = slice(128 * tt + 64 * h, 128 * tt + 64 * h + 64)
                nc.scalar.activation(x1f16[:, cols], kmx[64 * h:64 * h + 64, :],
                                     AF.Prelu, bias=w["b1b2"][64 * h:64 * h + 64, :],
                                     alpha=0.01)

        stA1 = conv_A(5, 64, w["w1mod"], w["w1bot"], w["b1a"],
                      fill_B_conv1, f5h)

        # ---- finish graph g-1 (conv2 B + lin1) while g's gathers fly ----
        if prev is not None:
            pst, psink, plin = prev
            conv_B(pst, w["w2b"], psink)
            lin1_pool(*plin)

        # ---- conv1 phase B, conv2 phase A ----
        conv_B(stA1, w["w1b"], sink1)

        def fill_B_conv2(B, x1f16=x1f16):
            nc.vector.tensor_copy(B[0:64, :], x1f16[:])
        x2f16 = sb.tile([64, N], F16, tag="x2f16")

        def sink2(tt, kmx, x2f16=x2f16):
            for h in range(2):
                cols = slice(128 * tt + 64 * h, 128 * tt + 64 * h + 64)
                nc.scalar.activation(x2f16[:, cols], kmx[64 * h:64 * h + 64, :],
                                     AF.Prelu, bias=w["b2b2"][64 * h:64 * h + 64, :],
                                     alpha=0.01)

        stA2 = conv_A(64, 128, w["w2mod"], w["w2bot"], w["b2a"],
                      fill_B_conv2, x1f16)
        prev = (stA2, sink2, (g, f5h, x1f16, x2f16))

    pst, psink, plin = prev
    conv_B(pst, w["w2b"], psink)
    lin1_pool(*plin)

    # ===================== head =====================
    t1p = ps_sm.tile([128, G], F32, tag="sm")
    for fo in range(2):
        gt = Gt_lo if fo == 0 else Gt_hi
        ga = sb.tile([128, G], F16, tag="ga")
        nc.scalar.activation(ga[:], gt[:], AF.Prelu, bias=w["bl2c"][:, fo:fo + 1],
                             alpha=0.01)
        nc.tensor.matmul(out=t1p[:], lhsT=w["wm1"][:, fo, :],
                         rhs=ga[:], start=(fo == 0), stop=(fo == 1))
    t1 = sb.tile([128, G], F16, tag="t1")
    nc.scalar.activation(t1[:], t1p[:], AF.Prelu, bias=w["bm1"][:], alpha=0.01)
    outp = ps_sm.tile([3, G], F32, tag="sm")
    nc.tensor.matmul(out=outp[:], lhsT=w["wm2"][:], rhs=t1[:], start=True, stop=True)
    outsb = sb.tile([3, G], F32, tag="outsb")
    nc.scalar.activation(outsb[:], outp[:], AF.Identity, bias=w["bm2"][:])
    nc.sync.dma_start(out=t["o"][:], in_=outsb[:])

    if rep_ctx is not None:
        rep_ctx.__exit__(None, None, None)

    for pool in (ps_sm, ps_a2, ps_a1, ps_nd, sbl, sbg, sb3, sb, sbw):
        pool.release()


# ======================= harness entry point =======================
_CACHE = {}


def _get_program(G):
    if "nc" not in _CACHE:
        nc = make_nc()
        build(nc, G)
        _CACHE["nc"] = nc
    return _CACHE["nc"]


def kernel(x, pos, tq, batch, w1a, b1a, w1b, b1b, w2a, b2a, w2b, b2b,
           wl1, bl1, wl2, bl2, wm1, bm1, wm2, bm2):
    """Full-input entry: shards graphs over 8 NeuronCores, returns [128, 3]."""
    from concourse.bass_utils import run_bass_kernel_spmd
    inputs = dict(x=np.asarray(x), pos=np.asarray(pos), tq=np.asarray(tq),
                  w1a=np.asarray(w1a), b1a=np.asarray(b1a),
                  w1b=np.asarray(w1b), b1b=np.asarray(b1b),
                  w2a=np.asarray(w2a), b2a=np.asarray(b2a),
                  w2b=np.asarray(w2b), b2b=np.asarray(b2b),
                  wl1=np.asarray(wl1), bl1=np.asarray(bl1),
                  wl2=np.asarray(wl2), bl2=np.asarray(bl2),
                  wm1=np.asarray(wm1), bm1=np.asarray(bm1),
                  wm2=np.asarray(wm2), bm2=np.asarray(bm2))
    NCORES = 8
    B_all = inputs["x"].shape[0] // N
    G = B_all // NCORES
    nc = _get_program(G)
    in_maps = [host_prep(inputs, G, c) for c in range(NCORES)]
    res = run_bass_kernel_spmd(nc, in_maps, core_ids=list(range(NCORES)))
    out = np.concatenate([res.results[c]["o"].T for c in range(NCORES)], axis=0)
    return out.astype(np.float32)


# revision 41
# speedup vs baseline: 1.0018x; 1.0018x over previous
"""DGCNN (nn_DGCNN_type1) Trainium2 Bass kernel — self-contained.

Strategy: data-parallel over the 128 graphs, 16 per NeuronCore across 8 cores.
Per graph: f16 kNN-score matmul (s = 2*f@f.T - |f_j|^2, row-constant dropped)
+ DVE max8/max_index/match_replace top-16 on f16 scores; edge MLP via the
q-trick: q = W_bot^T @ feat computed once per conv, transposed on-chip to a
node-major f16 table, and gathered per edge with SBUF-source dma_gather
(SBUF->SBUF, no HBM traffic); an identity matmul folds the gathered columns
into PSUM on top of the broadcast W_mod matmul. k-max folded through the
monotone LeakyReLU as a PSUM-side reduce; lin1 (hidden shared across both
output halves) + global-max-pool + MLP head on-chip.
"""

import numpy as np
import concourse.bacc as bacc
import concourse.mybir as mybir
from concourse.tile import TileContext
from concourse.masks import make_identity

F32, F16, I16, U16 = (mybir.dt.float32, mybir.dt.float16, mybir.dt.int16,
                      mybir.dt.uint16)
F32R = mybir.dt.float32r
AF = mybir.ActivationFunctionType
ALU = mybir.AluOpType
AX = mybir.AxisListType

N = 512
K = 16
# timing-bisect knobs (default = full kernel)
SKIP_TOPK = False
SKIP_GATHER = False
NUM_QUEUES = 1     # SWDGE queues; gathers round-robin across them
NSPLIT = 8         # gathers per conv (8192/NSPLIT idxs each)
NCHUNK = N // 128  # 4 row-chunks for the NxN score matrix


def make_nc():
    return bacc.Bacc(num_swdge_queues=NUM_QUEUES)


def host_prep(inputs, G, core):
    """Build the per-core in_map (numpy only: layout/dtype prep, no model math)."""
    f16 = np.float16
    x, pos, tq = inputs["x"], inputs["pos"], inputs["tq"]
    B_all = x.shape[0] // N
    xx = np.concatenate([tq, x, pos], axis=1).reshape(B_all, N, 5).astype(np.float32)
    sl = slice(core * G, (core + 1) * G)
    feat5h = np.ascontiguousarray(xx[sl].transpose(0, 2, 1)).astype(f16)

    w1a, w1b = inputs["w1a"], inputs["w1b"]
    w2a, w2b = inputs["w2a"], inputs["w2b"]
    wl1 = inputs["wl1"]

    return {
        "feat5h": feat5h,
        "w1mod": np.ascontiguousarray((w1a[0:5] - w1a[5:10]).astype(f16)),
        "w1bot": np.ascontiguousarray(w1a[5:10].astype(f16)),
        "w1b": np.ascontiguousarray(w1b.astype(f16)),
        "b1a": inputs["b1a"].reshape(64, 1).astype(np.float32),
        "b1b2": np.tile(inputs["b1b"], 2).reshape(128, 1).astype(np.float32),
        "w2mod": np.ascontiguousarray((w2a[0:64] - w2a[64:128]).astype(f16)),
        "w2bot": np.ascontiguousarray(w2a[64:128].astype(f16)),
        "w2b": np.ascontiguousarray(w2b.astype(f16)),
        "b2a": inputs["b2a"].reshape(128, 1).astype(np.float32),
        "b2b2": np.tile(inputs["b2b"], 2).reshape(128, 1).astype(np.float32),
        "wl1xx": np.ascontiguousarray(wl1[0:5].astype(f16)),
        "wl1x1": np.ascontiguousarray(wl1[5:69].astype(f16)),
        "wl1x2": np.ascontiguousarray(wl1[69:133].astype(f16)),
        "bl1c": np.ascontiguousarray(inputs["bl1"].reshape(4, 128).T.astype(np.float32)),
        "wl2": np.ascontiguousarray(inputs["wl2"].astype(f16).reshape(4, 128, 256).transpose(1, 0, 2)),
        "bl2c": np.ascontiguousarray(inputs["bl2"].reshape(2, 128).T.astype(np.float32)),
        "wm1": np.ascontiguousarray(inputs["wm1"].astype(f16).reshape(2, 128, 128).transpose(1, 0, 2)),
        "bm1": inputs["bm1"].reshape(128, 1).astype(np.float32),
        "wm2": np.ascontiguousarray(inputs["wm2"].astype(f16)),
        "bm2": inputs["bm2"].reshape(3, 1).astype(np.float32),
        "repl16": np.ascontiguousarray(np.tile(np.eye(16), 8).astype(f16)),
    }


def declare_io(nc, G):
    t = {}
    def inp(name, shape, dt):
        t[name] = nc.dram_tensor(name, shape, dt, kind="ExternalInput")
    inp("feat5h", [G, 5, N], F16)
    inp("w1mod", [5, 64], F16); inp("w1bot", [5, 64], F16)
    inp("w1b", [64, 64], F16); inp("b1a", [64, 1], F32); inp("b1b2", [128, 1], F32)
    inp("w2mod", [64, 128], F16); inp("w2bot", [64, 128], F16)
    inp("w2b", [128, 64], F16); inp("b2a", [128, 1], F32); inp("b2b2", [128, 1], F32)
    inp("wl1xx", [5, N], F16); inp("wl1x1", [64, N], F16); inp("wl1x2", [64, N], F16)
    inp("bl1c", [128, 4], F32); inp("wl2", [128, 4, 256], F16); inp("bl2c", [128, 2], F32)
    inp("wm1", [128, 2, 128], F16); inp("bm1", [128, 1], F32)
    inp("wm2", [128, 3], F16); inp("bm2", [3, 1], F32)
    inp("repl16", [16, 128], F16)
    t["o"] = nc.dram_tensor("o", [3, G], F32, kind="ExternalOutput")
    return t


def build(nc, G, reps=1):
    t = declare_io(nc, G)
    with TileContext(nc) as tc:
        _build_body(nc, tc, t, G, reps)
    nc.compile()
    return t


def _build_body(nc, tc, t, G, reps=1):
    sbw = tc.alloc_tile_pool(name="sbw", bufs=1)          # persistent
    sb = tc.alloc_tile_pool(name="sb", bufs=4)            # rotating tiles
    sb3 = tc.alloc_tile_pool(name="sb3", bufs=4)          # h1 rotation
    sbg = tc.alloc_tile_pool(name="sbg", bufs=2 * NSPLIT)  # in-flight gather outs
    sbl = tc.alloc_tile_pool(name="sbl", bufs=4)          # lin1 hidden cache
    ps_nd = tc.alloc_tile_pool(name="ps_nd", bufs=1, space="PSUM")   # 1 bank
    ps_a1 = tc.alloc_tile_pool(name="ps_a1", bufs=2, space="PSUM")   # 4 banks
    ps_a2 = tc.alloc_tile_pool(name="ps_a2", bufs=1, space="PSUM")   # 2 banks
    ps_sm = tc.alloc_tile_pool(name="ps_sm", bufs=1, space="PSUM")   # 1 bank

    # ---- persistent weight tiles ----
    w = {}
    for name in ["w1mod", "w1bot", "w1b", "w2mod", "w2bot", "w2b",
                 "wl1xx", "wl1x1", "wl1x2", "wl2", "wm1", "wm2", "repl16"]:
        w[name] = sbw.tile(list(t[name].shape), F16, tag=name, name='w_'+name)
        nc.sync.dma_start(out=w[name][:], in_=t[name][:])
    for name in ["b1a", "b1b2", "b2a", "b2b2", "bl1c", "bl2c", "bm1", "bm2"]:
        w[name] = sbw.tile(list(t[name].shape), F32, tag=name, name='b_'+name)
        nc.sync.dma_start(out=w[name][:], in_=t[name][:])
    ident = sbw.tile([128, 128], F16, tag="ident")
    make_identity(nc, ident[:])
    ones = sbw.tile([64, 1], F16, tag="ones")
    nc.gpsimd.memset(ones[:], 1.0)
    onesrow = sbw.tile([1, N], F16, tag="onesrow")
    nc.gpsimd.memset(onesrow[:], 1.0)

    # persistent gather-index tiles (all 128 rows hold the wrapped idxT,
    # replicated per 16-partition gpsimd core group)
    NIDX_SLOTS = 4
    idx_tiles = []
    for s in range(NIDX_SLOTS):
        it = sbw.tile([128, N], I16, tag=f"idxs{s}", name=f"idxs{s}")
        nc.gpsimd.memset(it[:], 0)
        idx_tiles.append(it)
    idx_slot = [0]
    gq = [0]  # round-robin SWDGE queue cursor for the gathers

    Gt_lo = sbw.tile([128, G], F32, tag="gtlo")
    Gt_hi = sbw.tile([128, G], F32, tag="gthi")

    rep_ctx = tc.For_i(0, reps, 1) if reps > 1 else None
    if rep_ctx is not None:
        rep_ctx.__enter__()

    def conv_A(d, Fmid, wmod, wbot, ba, fill_B, featTh):
        """DynamicEdgeConv phase A: scores, topk, index replication, q
        table, and the 4 SBUF-source gathers (launched, not consumed).
        Returns state for conv_B. Emitting other work between A and B
        hides the gather DMA latency."""
        dp = d + 1
        B = sb.tile([65, N], F16, tag="Btile")
        fill_B(B)                              # fills B[0:d, :] (f16)
        sc = sb.tile([65, N], F16, tag="sctile")
        nc.vector.tensor_scalar_mul(sc[0:d, :], B[0:d, :], 2.0)
        F2 = sb.tile([64, N], F16, tag="F2tile")
        nc.scalar.activation(F2[0:d, :], B[0:d, :], AF.Square)
        sqp = ps_sm.tile([1, N], F32, tag="sm")
        nc.tensor.matmul(out=sqp[:], lhsT=ones[0:d, :], rhs=F2[0:d, :],
                         start=True, stop=True)
        if d % 32 == 0:
            nc.gpsimd.memset(sc[d:d + 1, :], 1.0)
            nc.scalar.activation(B[d:d + 1, :], sqp[:], AF.Identity, scale=-1.0)
        else:
            nc.sync.dma_start(out=sc[d:d + 1, :], in_=onesrow[:])
            sqtmp = sb.tile([1, N], F16, tag="sqtmp")
            nc.scalar.activation(sqtmp[:], sqp[:], AF.Identity, scale=-1.0)
            nc.sync.dma_start(out=B[d:d + 1, :], in_=sqtmp[:])

        # ---- negd2 chunks + topk -> replicated idx tile ----
        idxs = idx_tiles[idx_slot[0] % NIDX_SLOTS]
        idx_slot[0] += 1
        idxTp = ps_sm.tile([16, N], F16, tag="sm")
        for c in range(NCHUNK):
            nd_p = ps_nd.tile([128, N], F32, tag="ndp")
            nc.tensor.matmul(out=nd_p[:], lhsT=sc[0:dp, 128 * c:128 * (c + 1)],
                             rhs=B[0:dp, :], start=True, stop=True)
            nd = sb.tile([128, N], F16, tag="ndsb")
            nc.scalar.activation(nd[:], nd_p[:], AF.Copy)
            maxv = sb.tile([128, 16], F16, tag="maxv")
            maxi = sb.tile([128, 16], U16, tag="maxi")
            if not SKIP_TOPK:
                nc.vector.max(out=maxv[:, 0:8], in_=nd[:])
                nc.vector.max_index(out=maxi[:, 0:8], in_max=maxv[:, 0:8], in_values=nd[:])
                nc.vector.match_replace(out=nd[:], in_to_replace=maxv[:, 0:8],
                                        in_values=nd[:], imm_value=-30000.0)
                nc.vector.max(out=maxv[:, 8:16], in_=nd[:])
                nc.vector.max_index(out=maxi[:, 8:16], in_max=maxv[:, 8:16], in_values=nd[:])
            else:
                nc.vector.memset(maxi[:], 0)
            mif = sb.tile([128, 16], F16, tag="mif")
            nc.vector.tensor_copy(mif[:], maxi[:])
            nc.tensor.transpose(out=idxTp[:, 128 * c:128 * (c + 1)], in_=mif[:],
                                identity=ident[:])
        # replicate wrapped idxT to all 8 gpsimd core groups via a
        # tiled-identity matmul: idxs[p, n] = idxT[p % 16, n]
        idxT16 = sb.tile([16, N], F16, tag="idxT16")
        nc.scalar.activation(idxT16[:], idxTp[:], AF.Copy)
        idxmm = ps_nd.tile([128, N], F32, tag="ndp")
        nc.tensor.matmul(out=idxmm[:], lhsT=w["repl16"][:], rhs=idxT16[:],
                         start=True, stop=True)
        nc.vector.tensor_copy(idxs[:], idxmm[:])

        # ---- q table: q = wbot^T @ feat, then node-major f16 for the
        # SBUF-source gather: T partition m%128, bytes [256*(m//128), +256)
        # hold node m's 128-row f16 q column ----
        qp = ps_nd.tile([128, N], F32, tag="ndp")
        nc.tensor.matmul(out=qp[0:Fmid, :], lhsT=wbot[:], rhs=featTh[:],
                         start=True, stop=True)
        qh = sb.tile([128, N], F16, tag="qtab")
        nc.scalar.activation(qh[0:Fmid, :], qp[0:Fmid, :], AF.Copy)
        if Fmid < 128:
            nc.gpsimd.memset(qh[Fmid:128, :], 0)
        Tp = ps_sm.tile([128, N], F16, tag="sm")
        for c in range(NCHUNK):
            nc.tensor.transpose(out=Tp[:, 128 * c:128 * (c + 1)],
                                in_=qh[:, 128 * c:128 * (c + 1)],
                                identity=ident[:])
        T = sb.tile([128, N], F16, tag="Ttab")
        nc.scalar.activation(T[:], Tp[:], AF.Copy)

        # ---- launch the gathers (consumed later in conv_B). Split into
        # NSPLIT pieces so each stays within the 128-deep SWDGE descriptor
        # ring (no desc-gen throttling on the Pool engine). ----
        GSZ = (K * N) // NSPLIT           # idxs per gather
        qgs = []
        for u in range(NSPLIT):
            qg = sbg.tile([128, GSZ], F16, tag="qg")
            if SKIP_GATHER:
                for z in range(GSZ // 512):
                    nc.scalar.activation(qg[:, 512 * z:512 * (z + 1)], T[:],
                                         AF.Copy)
            else:
                nc.gpsimd.dma_gather(
                    out_ap=qg[:, None, :], in_ap=T[:],
                    idxs_ap=idxs[:, (GSZ // 16) * u:(GSZ // 16) * (u + 1)],
                    num_idxs=GSZ, num_idxs_reg=GSZ, elem_size=128,
                    transpose=True, single_packet=False,
                    queue_num=gq[0] % NUM_QUEUES,
                    sbuf_tokens_per_rank=128, sbuf_free_dim_per_rank=256,
                    sbuf_free_dim_pad_per_rank=0, sbuf_byte_offset=0)
                gq[0] += 1
            qgs.append(qg)
        return dict(d=d, Fmid=Fmid, wmod=wmod, ba=ba, featTh=featTh, qgs=qgs)

    def conv_B(st, wsec, kact_sink):
        """DynamicEdgeConv phase B: fold gathered q into the broadcast
        W_mod matmul, Prelu, layer2, k-max. kact_sink(tt, kmx): kmx
        [128, 64] f32 pre-activation k-max, rows 64h+f = feature f of
        node group h; tile tt covers nodes [128*tt + 64*h, +64); the
        sink applies the output Prelu itself while scattering halves."""
        d, Fmid, wmod, ba, featTh, qgs = (st["d"], st["Fmid"], st["wmod"],
                                          st["ba"], st["featTh"], st["qgs"])
        CPG = 16 // NSPLIT                # 512-col chunks per gather tile
        for tt in range(4):
            a2 = ps_a2.tile([128, 1024], F32, tag="a2")
            for hh in range(2):
                a1 = ps_a1.tile([128, 1024], F32, tag="a1")
                for r in range(2):
                    qq = 2 * hh + r
                    c = 4 * tt + qq
                    nc.tensor.matmul(out=a1[0:Fmid, 512 * r:512 * (r + 1)],
                                     lhsT=wmod[:],
                                     rhs=featTh[:, 32 * c:32 * (c + 1), None]
                                     .to_broadcast([d, 32, K]),
                                     start=True, stop=False)
                    c = 4 * tt + qq
                    nc.tensor.matmul(out=a1[0:Fmid, 512 * r:512 * (r + 1)],
                                     lhsT=ident[:, 0:Fmid],
                                     rhs=qgs[c // CPG][:, 512 * (c % CPG):
                                                       512 * (c % CPG) + 512],
                                     start=False, stop=True)
                h1 = sb3.tile([128, 1024], F16, tag="h1")
                nc.scalar.activation(h1[0:Fmid, :], a1[0:Fmid, :], AF.Prelu,
                                     bias=ba[:], alpha=0.01)
                for r in range(2):
                    nc.tensor.matmul(out=a2[64 * hh:64 * hh + 64, 512 * r:512 * (r + 1)],
                                     lhsT=wsec[:], rhs=h1[0:Fmid, 512 * r:512 * (r + 1)],
                                     start=True, stop=True)
            kmx = sb.tile([128, 64], F32, tag="kmx")
            nc.vector.tensor_reduce(out=kmx[:], in_=a2[:].rearrange(
                "p (n k) -> p n k", k=K), op=ALU.max, axis=AX.X)
            kact_sink(tt, kmx)

    def lin1_pool(g, f5h, x1f16, x2f16):
        hsbs = []
        for c in range(NCHUNK):
            hp = ps_a1.tile([128, 1024], F32, tag="a1")
            nc.tensor.matmul(out=hp[:, 0:N], lhsT=w["wl1xx"][:, 128 * c:128 * (c + 1)],
                             rhs=f5h[:], start=True, stop=False)
            nc.tensor.matmul(out=hp[:, 0:N], lhsT=w["wl1x1"][:, 128 * c:128 * (c + 1)],
                             rhs=x1f16[:], start=False, stop=False)
            nc.tensor.matmul(out=hp[:, 0:N], lhsT=w["wl1x2"][:, 128 * c:128 * (c + 1)],
                             rhs=x2f16[:], start=False, stop=True)
            hsb = sbl.tile([128, N], F16, tag="hsb")
            nc.scalar.activation(hsb[:], hp[:, 0:N], AF.Prelu,
                                 bias=w["bl1c"][:, c:c + 1], alpha=0.01)
            hsbs.append(hsb)
        for fo in range(2):
            h2p = ps_a2.tile([128, 1024], F32, tag="a2")
            for c in range(NCHUNK):
                nc.tensor.matmul(out=h2p[:, 0:N],
                                 lhsT=w["wl2"][:, c, 128 * fo:128 * (fo + 1)],
                                 rhs=hsbs[c][:], start=(c == 0), stop=(c == NCHUNK - 1))
            gt = Gt_lo if fo == 0 else Gt_hi
            nc.vector.tensor_reduce(out=gt[:, g:g + 1], in_=h2p[:, 0:N], op=ALU.max,
                                    axis=AX.X)

    # Software-pipelined graph loop: each conv's gathers (phase A) are
    # launched one work-unit ahead of their consumption (phase B), so the
    # SWDGE DMA latency hides under the neighbouring unit's compute.
    prev = None  # (stA2, sink2, lin1 args) for graph g-1
    for g in range(G):
        # ---- conv1 phase A ----
        f5h = sb.tile([5, N], F16, tag="f5h")
        nc.sync.dma_start(out=f5h[:], in_=t["feat5h"][g])

        def fill_B_conv1(B, g=g):
            nc.sync.dma_start(out=B[0:5, :], in_=t["feat5h"][g])
        x1f16 = sb.tile([64, N], F16, tag="x1f16")

        def sink1(tt, kmx, x1f16=x1f16):
            for h in range(2):
                cols = slice(128 * tt + 64 * h, 128 * tt + 64 * h + 64)
                nc.scalar.activation(x1f16[:, cols], kmx[64 * h:64 * h + 64, :],
                                     AF.Prelu, bias=w["b1b2"][64 * h:64 * h + 64, :],
                                     alpha=0.01)

        stA1 = conv_A(5, 64, w["w1mod"], w["w1bot"], w["b1a"],
                      fill_B_conv1, f5h)

        # ---- finish graph g-1 (conv2 B + lin1) while g's gathers fly ----
        if prev is not None:
            pst, psink, plin = prev
            conv_B(pst, w["w2b"], psink)
            lin1_pool(*plin)

        # ---- conv1 phase B, conv2 phase A ----
        conv_B(stA1, w["w1b"], sink1)

        def fill_B_conv2(B, x1f16=x1f16):
            nc.vector.tensor_copy(B[0:64, :], x1f16[:])
        x2f16 = sb.tile([64, N], F16, tag="x2f16")

        def sink2(tt, kmx, x2f16=x2f16):
            for h in range(2):
                cols = slice(128 * tt + 64 * h, 128 * tt + 64 * h + 64)
                nc.scalar.activation(x2f16[:, cols], kmx[64 * h:64 * h + 64, :],
                                     AF.Prelu, bias=w["b2b2"][64 * h:64 * h + 64, :],
                                     alpha=0.01)

        stA2 = conv_A(64, 128, w["w2mod"], w["w2bot"], w["b2a"],
                      fill_B_conv2, x1f16)
        prev = (stA2, sink2, (g, f5h, x1f16, x2f16))

    pst, psink, plin = prev
    conv_B(pst, w["w2b"], psink)
    lin1_pool(*plin)

    # ===================== head =====================
    t1p = ps_sm.tile([128, G], F32, tag="sm")
    for fo in range(2):
        gt = Gt_lo if fo == 0 else Gt_hi
        ga = sb.tile([128, G], F16, tag="ga")
        nc.scalar.activation(ga[:], gt[:], AF.Prelu, bias=w["bl2c"][:, fo:fo + 1],
                             alpha=0.01)
        nc.tensor.matmul(out=t1p[:], lhsT=w["wm1"][:, fo, :],
                         rhs=ga[:], start=(fo == 0), stop=(fo == 1))
    t1 = sb.tile([128, G], F16, tag="t1")
    nc.scalar.activation(t1[:], t1p[:], AF.Prelu, bias=w["bm1"][:], alpha=0.01)
    outp = ps_sm.tile([3, G], F32, tag="sm")
    nc.tensor.matmul(out=outp[:], lhsT=w["wm2"][:], rhs=t1[:], start=True, stop=True)
    outsb = sb.tile([3, G], F32, tag="outsb")
    nc.scalar.activation(outsb[:], outp[:], AF.Identity, bias=w["bm2"][:])
    nc.sync.dma_start(out=t["o"][:], in_=outsb[:])

    if rep_ctx is not None:
        rep_ctx.__exit__(None, None, None)

    for pool in (ps_sm, ps_a2, ps_a1, ps_nd, sbl, sbg, sb3, sb, sbw):
        pool.release()


# ======================= harness entry point =======================
_CACHE = {}


def _get_program(G):
    if "nc" not in _CACHE:
        nc = make_nc()
        build(nc, G)
        _CACHE["nc"] = nc
    return _CACHE["nc"]


def kernel(x, pos, tq, batch, w1a, b1a, w1b, b1b, w2a, b2a, w2b, b2b,
           wl1, bl1, wl2, bl2, wm1, bm1, wm2, bm2):
    """Full-input entry: shards graphs over 8 NeuronCores, returns [128, 3]."""
    from concourse.bass_utils import run_bass_kernel_spmd
    inputs = dict(x=np.asarray(x), pos=np.asarray(pos), tq=np.asarray(tq),
                  w1a=np.asarray(w1a), b1a=np.asarray(b1a),
                  w1b=np.asarray(w1b), b1b=np.asarray(b1b),
                  w2a=np.asarray(w2a), b2a=np.asarray(b2a),
                  w2b=np.asarray(w2b), b2b=np.asarray(b2b),
                  wl1=np.asarray(wl1), bl1=np.asarray(bl1),
                  wl2=np.asarray(wl2), bl2=np.asarray(bl2),
                  wm1=np.asarray(wm1), bm1=np.asarray(bm1),
                  wm2=np.asarray(wm2), bm2=np.asarray(bm2))
    NCORES = 8
    B_all = inputs["x"].shape[0] // N
    G = B_all // NCORES
    nc = _get_program(G)
    in_maps = [host_prep(inputs, G, c) for c in range(NCORES)]
    res = run_bass_kernel_spmd(nc, in_maps, core_ids=list(range(NCORES)))
    out = np.concatenate([res.results[c]["o"].T for c in range(NCORES)], axis=0)
    return out.astype(np.float32)


# revision 42
# speedup vs baseline: 1.1335x; 1.1315x over previous
"""DGCNN (nn_DGCNN_type1) Trainium2 Bass kernel — self-contained.

Strategy: data-parallel over the 128 graphs, 16 per NeuronCore across 8 cores.
Per graph: f16 kNN-score matmul (s = 2*f@f.T - |f_j|^2, row-constant dropped)
+ DVE max8/max_index/match_replace top-16 on f16 scores; edge MLP via the
q-trick: q = W_bot^T @ feat computed once per conv, transposed on-chip to a
node-major f16 table, and gathered per edge with SBUF-source dma_gather
(SBUF->SBUF, no HBM traffic); an identity matmul folds the gathered columns
into PSUM on top of the broadcast W_mod matmul. k-max folded through the
monotone LeakyReLU as a PSUM-side reduce; lin1 (hidden shared across both
output halves) + global-max-pool + MLP head on-chip.
"""

import numpy as np
import concourse.bacc as bacc
import concourse.mybir as mybir
from concourse.tile import TileContext
from concourse.masks import make_identity

F32, F16, I16, U16 = (mybir.dt.float32, mybir.dt.float16, mybir.dt.int16,
                      mybir.dt.uint16)
F32R = mybir.dt.float32r
AF = mybir.ActivationFunctionType
ALU = mybir.AluOpType
AX = mybir.AxisListType

N = 512
K = 16
# timing-bisect knobs (default = full kernel)
SKIP_TOPK = False
SKIP_GATHER = False
NUM_QUEUES = 1     # SWDGE queues; gathers round-robin across them
NSPLIT = 8         # gathers per conv (8192/NSPLIT idxs each)
NCHUNK = N // 128  # 4 row-chunks for the NxN score matrix


def make_nc():
    return bacc.Bacc(num_swdge_queues=NUM_QUEUES)


def host_prep(inputs, G, core):
    """Build the per-core in_map (numpy only: layout/dtype prep, no model math)."""
    f16 = np.float16
    x, pos, tq = inputs["x"], inputs["pos"], inputs["tq"]
    B_all = x.shape[0] // N
    xx = np.concatenate([tq, x, pos], axis=1).reshape(B_all, N, 5).astype(np.float32)
    sl = slice(core * G, (core + 1) * G)
    feat5h = np.ascontiguousarray(xx[sl].transpose(0, 2, 1)).astype(f16)

    w1a, w1b = inputs["w1a"], inputs["w1b"]
    w2a, w2b = inputs["w2a"], inputs["w2b"]
    wl1 = inputs["wl1"]

    return {
        "feat5h": feat5h,
        "w1mod": np.ascontiguousarray((w1a[0:5] - w1a[5:10]).astype(f16)),
        "w1bot": np.ascontiguousarray(w1a[5:10].astype(f16)),
        "w1b": np.ascontiguousarray(w1b.astype(f16)),
        "b1a": inputs["b1a"].reshape(64, 1).astype(np.float32),
        "b1b2": np.tile(inputs["b1b"], 2).reshape(128, 1).astype(np.float32),
        "w2mod": np.ascontiguousarray((w2a[0:64] - w2a[64:128]).astype(f16)),
        "w2bot": np.ascontiguousarray(w2a[64:128].astype(f16)),
        "w2b": np.ascontiguousarray(w2b.astype(f16)),
        "b2a": inputs["b2a"].reshape(128, 1).astype(np.float32),
        "b2b2": np.tile(inputs["b2b"], 2).reshape(128, 1).astype(np.float32),
        "wl1xx": np.ascontiguousarray(wl1[0:5].astype(f16)),
        "wl1x1": np.ascontiguousarray(wl1[5:69].astype(f16)),
        "wl1x2": np.ascontiguousarray(wl1[69:133].astype(f16)),
        "bl1c": np.ascontiguousarray(inputs["bl1"].reshape(4, 128).T.astype(np.float32)),
        "wl2": np.ascontiguousarray(inputs["wl2"].astype(f16).reshape(4, 128, 256).transpose(1, 0, 2)),
        "bl2c": np.ascontiguousarray(inputs["bl2"].reshape(2, 128).T.astype(np.float32)),
        "wm1": np.ascontiguousarray(inputs["wm1"].astype(f16).reshape(2, 128, 128).transpose(1, 0, 2)),
        "bm1": inputs["bm1"].reshape(128, 1).astype(np.float32),
        "wm2": np.ascontiguousarray(inputs["wm2"].astype(f16)),
        "bm2": inputs["bm2"].reshape(3, 1).astype(np.float32),
        "repl16": np.ascontiguousarray(np.tile(np.eye(16), 8).astype(f16)),
    }


def declare_io(nc, G):
    t = {}
    def inp(name, shape, dt):
        t[name] = nc.dram_tensor(name, shape, dt, kind="ExternalInput")
    inp("feat5h", [G, 5, N], F16)
    inp("w1mod", [5, 64], F16); inp("w1bot", [5, 64], F16)
    inp("w1b", [64, 64], F16); inp("b1a", [64, 1], F32); inp("b1b2", [128, 1], F32)
    inp("w2mod", [64, 128], F16); inp("w2bot", [64, 128], F16)
    inp("w2b", [128, 64], F16); inp("b2a", [128, 1], F32); inp("b2b2", [128, 1], F32)
    inp("wl1xx", [5, N], F16); inp("wl1x1", [64, N], F16); inp("wl1x2", [64, N], F16)
    inp("bl1c", [128, 4], F32); inp("wl2", [128, 4, 256], F16); inp("bl2c", [128, 2], F32)
    inp("wm1", [128, 2, 128], F16); inp("bm1", [128, 1], F32)
    inp("wm2", [128, 3], F16); inp("bm2", [3, 1], F32)
    inp("repl16", [16, 128], F16)
    t["o"] = nc.dram_tensor("o", [3, G], F32, kind="ExternalOutput")
    return t


def build(nc, G, reps=1):
    t = declare_io(nc, G)
    with TileContext(nc) as tc:
        _build_body(nc, tc, t, G, reps)
    nc.compile()
    return t


def _build_body(nc, tc, t, G, reps=1):
    sbw = tc.alloc_tile_pool(name="sbw", bufs=1)          # persistent
    sb = tc.alloc_tile_pool(name="sb", bufs=4)            # rotating tiles
    sb3 = tc.alloc_tile_pool(name="sb3", bufs=4)          # h1 rotation
    sbg = tc.alloc_tile_pool(name="sbg", bufs=2 * NSPLIT)  # in-flight gather outs
    sbl = tc.alloc_tile_pool(name="sbl", bufs=4)          # lin1 hidden cache
    ps_nd = tc.alloc_tile_pool(name="ps_nd", bufs=2, space="PSUM")   # 2 banks
    ps_a1 = tc.alloc_tile_pool(name="ps_a1", bufs=2, space="PSUM")   # 2 banks
    ps_a2 = tc.alloc_tile_pool(name="ps_a2", bufs=1, space="PSUM")   # 2 banks
    ps_sm = tc.alloc_tile_pool(name="ps_sm", bufs=2, space="PSUM")   # 2 banks

    # ---- persistent weight tiles ----
    w = {}
    for name in ["w1mod", "w1bot", "w1b", "w2mod", "w2bot", "w2b",
                 "wl1xx", "wl1x1", "wl1x2", "wl2", "wm1", "wm2", "repl16"]:
        w[name] = sbw.tile(list(t[name].shape), F16, tag=name, name='w_'+name)
        nc.sync.dma_start(out=w[name][:], in_=t[name][:])
    for name in ["b1a", "b1b2", "b2a", "b2b2", "bl1c", "bl2c", "bm1", "bm2"]:
        w[name] = sbw.tile(list(t[name].shape), F32, tag=name, name='b_'+name)
        nc.sync.dma_start(out=w[name][:], in_=t[name][:])
    ident = sbw.tile([128, 128], F16, tag="ident")
    make_identity(nc, ident[:])
    ones = sbw.tile([64, 1], F16, tag="ones")
    nc.gpsimd.memset(ones[:], 1.0)
    onesrow = sbw.tile([1, N], F16, tag="onesrow")
    nc.gpsimd.memset(onesrow[:], 1.0)

    # persistent gather-index tiles (all 128 rows hold the wrapped idxT,
    # replicated per 16-partition gpsimd core group)
    NIDX_SLOTS = 4
    idx_tiles = []
    for s in range(NIDX_SLOTS):
        it = sbw.tile([128, N], I16, tag=f"idxs{s}", name=f"idxs{s}")
        nc.gpsimd.memset(it[:], 0)
        idx_tiles.append(it)
    idx_slot = [0]
    gq = [0]  # round-robin SWDGE queue cursor for the gathers

    Gt_lo = sbw.tile([128, G], F32, tag="gtlo")
    Gt_hi = sbw.tile([128, G], F32, tag="gthi")

    rep_ctx = tc.For_i(0, reps, 1) if reps > 1 else None
    if rep_ctx is not None:
        rep_ctx.__enter__()

    def conv_A(d, Fmid, wmod, wbot, ba, fill_B, featTh):
        """DynamicEdgeConv phase A: scores, topk, index replication, q
        table, and the 4 SBUF-source gathers (launched, not consumed).
        Returns state for conv_B. Emitting other work between A and B
        hides the gather DMA latency."""
        dp = d + 1
        B = sb.tile([65, N], F16, tag="Btile")
        fill_B(B)                              # fills B[0:d, :] (f16)
        sc = sb.tile([65, N], F16, tag="sctile")
        nc.vector.tensor_scalar_mul(sc[0:d, :], B[0:d, :], 2.0)
        F2 = sb.tile([64, N], F16, tag="F2tile")
        nc.scalar.activation(F2[0:d, :], B[0:d, :], AF.Square)
        sqp = ps_sm.tile([1, N], F32, tag="sm")
        nc.tensor.matmul(out=sqp[:], lhsT=ones[0:d, :], rhs=F2[0:d, :],
                         start=True, stop=True)
        if d % 32 == 0:
            nc.gpsimd.memset(sc[d:d + 1, :], 1.0)
            nc.scalar.activation(B[d:d + 1, :], sqp[:], AF.Identity, scale=-1.0)
        else:
            nc.sync.dma_start(out=sc[d:d + 1, :], in_=onesrow[:])
            sqtmp = sb.tile([1, N], F16, tag="sqtmp")
            nc.scalar.activation(sqtmp[:], sqp[:], AF.Identity, scale=-1.0)
            nc.sync.dma_start(out=B[d:d + 1, :], in_=sqtmp[:])

        # ---- negd2 chunks + topk -> replicated idx tile ----
        idxs = idx_tiles[idx_slot[0] % NIDX_SLOTS]
        idx_slot[0] += 1
        idxTp = ps_sm.tile([16, N], F16, tag="sm")
        for c in range(NCHUNK):
            nd_p = ps_nd.tile([128, N], F32, tag="ndp")
            nc.tensor.matmul(out=nd_p[:], lhsT=sc[0:dp, 128 * c:128 * (c + 1)],
                             rhs=B[0:dp, :], start=True, stop=True)
            nd = sb.tile([128, N], F16, tag="ndsb")
            nc.scalar.activation(nd[:], nd_p[:], AF.Copy)
            maxv = sb.tile([128, 16], F16, tag="maxv")
            maxi = sb.tile([128, 16], U16, tag="maxi")
            if not SKIP_TOPK:
                nc.vector.max(out=maxv[:, 0:8], in_=nd[:])
                nc.vector.max_index(out=maxi[:, 0:8], in_max=maxv[:, 0:8], in_values=nd[:])
                nc.vector.match_replace(out=nd[:], in_to_replace=maxv[:, 0:8],
                                        in_values=nd[:], imm_value=-30000.0)
                nc.vector.max(out=maxv[:, 8:16], in_=nd[:])
                nc.vector.max_index(out=maxi[:, 8:16], in_max=maxv[:, 8:16], in_values=nd[:])
            else:
                nc.vector.memset(maxi[:], 0)
            mif = sb.tile([128, 16], F16, tag="mif")
            nc.vector.tensor_copy(mif[:], maxi[:])
            nc.tensor.transpose(out=idxTp[:, 128 * c:128 * (c + 1)], in_=mif[:],
                                identity=ident[:])
        # replicate wrapped idxT to all 8 gpsimd core groups via a
        # tiled-identity matmul: idxs[p, n] = idxT[p % 16, n]
        idxT16 = sb.tile([16, N], F16, tag="idxT16")
        nc.scalar.activation(idxT16[:], idxTp[:], AF.Copy)
        idxmm = ps_nd.tile([128, N], F32, tag="ndp")
        nc.tensor.matmul(out=idxmm[:], lhsT=w["repl16"][:], rhs=idxT16[:],
                         start=True, stop=True)
        nc.vector.tensor_copy(idxs[:], idxmm[:])

        # ---- q table: q = wbot^T @ feat, then node-major f16 for the
        # SBUF-source gather: T partition m%128, bytes [256*(m//128), +256)
        # hold node m's 128-row f16 q column ----
        qp = ps_nd.tile([128, N], F32, tag="ndp")
        nc.tensor.matmul(out=qp[0:Fmid, :], lhsT=wbot[:], rhs=featTh[:],
                         start=True, stop=True)
        qh = sb.tile([128, N], F16, tag="qtab")
        nc.scalar.activation(qh[0:Fmid, :], qp[0:Fmid, :], AF.Copy)
        if Fmid < 128:
            nc.gpsimd.memset(qh[Fmid:128, :], 0)
        Tp = ps_sm.tile([128, N], F16, tag="sm")
        for c in range(NCHUNK):
            nc.tensor.transpose(out=Tp[:, 128 * c:128 * (c + 1)],
                                in_=qh[:, 128 * c:128 * (c + 1)],
                                identity=ident[:])
        T = sb.tile([128, N], F16, tag="Ttab")
        nc.scalar.activation(T[:], Tp[:], AF.Copy)

        # ---- launch the gathers (consumed later in conv_B). Split into
        # NSPLIT pieces so each stays within the 128-deep SWDGE descriptor
        # ring (no desc-gen throttling on the Pool engine). ----
        GSZ = (K * N) // NSPLIT           # idxs per gather
        qgs = []
        for u in range(NSPLIT):
            qg = sbg.tile([128, GSZ], F16, tag="qg")
            if SKIP_GATHER:
                for z in range(GSZ // 512):
                    nc.scalar.activation(qg[:, 512 * z:512 * (z + 1)], T[:],
                                         AF.Copy)
            else:
                nc.gpsimd.dma_gather(
                    out_ap=qg[:, None, :], in_ap=T[:],
                    idxs_ap=idxs[:, (GSZ // 16) * u:(GSZ // 16) * (u + 1)],
                    num_idxs=GSZ, num_idxs_reg=GSZ, elem_size=128,
                    transpose=True, single_packet=False,
                    queue_num=gq[0] % NUM_QUEUES,
                    sbuf_tokens_per_rank=128, sbuf_free_dim_per_rank=256,
                    sbuf_free_dim_pad_per_rank=0, sbuf_byte_offset=0)
                gq[0] += 1
            qgs.append(qg)
        return dict(d=d, Fmid=Fmid, wmod=wmod, ba=ba, featTh=featTh, qgs=qgs)

    def conv_B(st, wsec, kact_sink):
        """DynamicEdgeConv phase B: fold gathered q into the broadcast
        W_mod matmul, Prelu, layer2, k-max. kact_sink(tt, kmx): kmx
        [128, 64] f32 pre-activation k-max, rows 64h+f = feature f of
        node group h; tile tt covers nodes [128*tt + 64*h, +64); the
        sink applies the output Prelu itself while scattering halves."""
        d, Fmid, wmod, ba, featTh, qgs = (st["d"], st["Fmid"], st["wmod"],
                                          st["ba"], st["featTh"], st["qgs"])
        CPG = 16 // NSPLIT                # 512-col chunks per gather tile
        for tt in range(4):
            a2 = ps_a2.tile([128, 1024], F32, tag="a2")
            for qq in range(4):
                hh, r = qq // 2, qq % 2
                c = 4 * tt + qq
                a1 = ps_a1.tile([128, N], F32, tag="a1")
                nc.tensor.matmul(out=a1[0:Fmid, :],
                                 lhsT=wmod[:],
                                 rhs=featTh[:, 32 * c:32 * (c + 1), None]
                                 .to_broadcast([d, 32, K]),
                                 start=True, stop=False)
                nc.tensor.matmul(out=a1[0:Fmid, :],
                                 lhsT=ident[:, 0:Fmid],
                                 rhs=qgs[c // CPG][:, 512 * (c % CPG):
                                                   512 * (c % CPG) + 512],
                                 start=False, stop=True)
                h1 = sb3.tile([128, N], F16, tag="h1")
                nc.scalar.activation(h1[0:Fmid, :], a1[0:Fmid, :], AF.Prelu,
                                     bias=ba[:], alpha=0.01)
                nc.tensor.matmul(out=a2[64 * hh:64 * hh + 64, 512 * r:512 * (r + 1)],
                                 lhsT=wsec[:], rhs=h1[0:Fmid, :],
                                 start=True, stop=True)
            kmx = sb.tile([128, 64], F32, tag="kmx")
            nc.vector.tensor_reduce(out=kmx[:], in_=a2[:].rearrange(
                "p (n k) -> p n k", k=K), op=ALU.max, axis=AX.X)
            kact_sink(tt, kmx)

    def lin1_pool(g, f5h, x1f16, x2f16):
        hsbs = []
        for c in range(NCHUNK):
            hp = ps_a1.tile([128, N], F32, tag="a1")
            nc.tensor.matmul(out=hp[:], lhsT=w["wl1xx"][:, 128 * c:128 * (c + 1)],
                             rhs=f5h[:], start=True, stop=False)
            nc.tensor.matmul(out=hp[:], lhsT=w["wl1x1"][:, 128 * c:128 * (c + 1)],
                             rhs=x1f16[:], start=False, stop=False)
            nc.tensor.matmul(out=hp[:], lhsT=w["wl1x2"][:, 128 * c:128 * (c + 1)],
                             rhs=x2f16[:], start=False, stop=True)
            hsb = sbl.tile([128, N], F16, tag="hsb")
            nc.scalar.activation(hsb[:], hp[:], AF.Prelu,
                                 bias=w["bl1c"][:, c:c + 1], alpha=0.01)
            hsbs.append(hsb)
        for fo in range(2):
            h2p = ps_a2.tile([128, 1024], F32, tag="a2")
            for c in range(NCHUNK):
                nc.tensor.matmul(out=h2p[:, 0:N],
                                 lhsT=w["wl2"][:, c, 128 * fo:128 * (fo + 1)],
                                 rhs=hsbs[c][:], start=(c == 0), stop=(c == NCHUNK - 1))
            gt = Gt_lo if fo == 0 else Gt_hi
            nc.vector.tensor_reduce(out=gt[:, g:g + 1], in_=h2p[:, 0:N], op=ALU.max,
                                    axis=AX.X)

    # Software-pipelined graph loop: each conv's gathers (phase A) are
    # launched one work-unit ahead of their consumption (phase B), so the
    # SWDGE DMA latency hides under the neighbouring unit's compute.
    prev = None  # (stA2, sink2, lin1 args) for graph g-1
    for g in range(G):
        # ---- conv1 phase A ----
        f5h = sb.tile([5, N], F16, tag="f5h")
        nc.sync.dma_start(out=f5h[:], in_=t["feat5h"][g])

        def fill_B_conv1(B, g=g):
            nc.sync.dma_start(out=B[0:5, :], in_=t["feat5h"][g])
        x1f16 = sb.tile([64, N], F16, tag="x1f16")

        def sink1(tt, kmx, x1f16=x1f16):
            for h in range(2):
                cols = slice(128 * tt + 64 * h, 128 * tt + 64 * h + 64)
                nc.scalar.activation(x1f16[:, cols], kmx[64 * h:64 * h + 64, :],
                                     AF.Prelu, bias=w["b1b2"][64 * h:64 * h + 64, :],
                                     alpha=0.01)

        stA1 = conv_A(5, 64, w["w1mod"], w["w1bot"], w["b1a"],
                      fill_B_conv1, f5h)

        # ---- finish graph g-1 (conv2 B + lin1) while g's gathers fly ----
        if prev is not None:
            pst, psink, plin = prev
            conv_B(pst, w["w2b"], psink)
            lin1_pool(*plin)

        # ---- conv1 phase B, conv2 phase A ----
        conv_B(stA1, w["w1b"], sink1)

        def fill_B_conv2(B, x1f16=x1f16):
            nc.vector.tensor_copy(B[0:64, :], x1f16[:])
        x2f16 = sb.tile([64, N], F16, tag="x2f16")

        def sink2(tt, kmx, x2f16=x2f16):
            for h in range(2):
                cols = slice(128 * tt + 64 * h, 128 * tt + 64 * h + 64)
                nc.scalar.activation(x2f16[:, cols], kmx[64 * h:64 * h + 64, :],
                                     AF.Prelu, bias=w["b2b2"][64 * h:64 * h + 64, :],
                                     alpha=0.01)

        stA2 = conv_A(64, 128, w["w2mod"], w["w2bot"], w["b2a"],
                      fill_B_conv2, x1f16)
        prev = (stA2, sink2, (g, f5h, x1f16, x2f16))

    pst, psink, plin = prev
    conv_B(pst, w["w2b"], psink)
    lin1_pool(*plin)

    # ===================== head =====================
    t1p = ps_sm.tile([128, G], F32, tag="sm")
    for fo in range(2):
        gt = Gt_lo if fo == 0 else Gt_hi
        ga = sb.tile([128, G], F16, tag="ga")
        nc.scalar.activation(ga[:], gt[:], AF.Prelu, bias=w["bl2c"][:, fo:fo + 1],
                             alpha=0.01)
        nc.tensor.matmul(out=t1p[:], lhsT=w["wm1"][:, fo, :],
                         rhs=ga[:], start=(fo == 0), stop=(fo == 1))
    t1 = sb.tile([128, G], F16, tag="t1")
    nc.scalar.activation(t1[:], t1p[:], AF.Prelu, bias=w["bm1"][:], alpha=0.01)
    outp = ps_sm.tile([3, G], F32, tag="sm")
    nc.tensor.matmul(out=outp[:], lhsT=w["wm2"][:], rhs=t1[:], start=True, stop=True)
    outsb = sb.tile([3, G], F32, tag="outsb")
    nc.scalar.activation(outsb[:], outp[:], AF.Identity, bias=w["bm2"][:])
    nc.sync.dma_start(out=t["o"][:], in_=outsb[:])

    if rep_ctx is not None:
        rep_ctx.__exit__(None, None, None)

    for pool in (ps_sm, ps_a2, ps_a1, ps_nd, sbl, sbg, sb3, sb, sbw):
        pool.release()


# ======================= harness entry point =======================
_CACHE = {}


def _get_program(G):
    if "nc" not in _CACHE:
        nc = make_nc()
        build(nc, G)
        _CACHE["nc"] = nc
    return _CACHE["nc"]


def kernel(x, pos, tq, batch, w1a, b1a, w1b, b1b, w2a, b2a, w2b, b2b,
           wl1, bl1, wl2, bl2, wm1, bm1, wm2, bm2):
    """Full-input entry: shards graphs over 8 NeuronCores, returns [128, 3]."""
    from concourse.bass_utils import run_bass_kernel_spmd
    inputs = dict(x=np.asarray(x), pos=np.asarray(pos), tq=np.asarray(tq),
                  w1a=np.asarray(w1a), b1a=np.asarray(b1a),
                  w1b=np.asarray(w1b), b1b=np.asarray(b1b),
                  w2a=np.asarray(w2a), b2a=np.asarray(b2a),
                  w2b=np.asarray(w2b), b2b=np.asarray(b2b),
                  wl1=np.asarray(wl1), bl1=np.asarray(bl1),
                  wl2=np.asarray(wl2), bl2=np.asarray(bl2),
                  wm1=np.asarray(wm1), bm1=np.asarray(bm1),
                  wm2=np.asarray(wm2), bm2=np.asarray(bm2))
    NCORES = 8
    B_all = inputs["x"].shape[0] // N
    G = B_all // NCORES
    nc = _get_program(G)
    in_maps = [host_prep(inputs, G, c) for c in range(NCORES)]
    res = run_bass_kernel_spmd(nc, in_maps, core_ids=list(range(NCORES)))
    out = np.concatenate([res.results[c]["o"].T for c in range(NCORES)], axis=0)
    return out.astype(np.float32)
